# revision 15
# baseline (speedup 1.0000x reference)
"""GCN (3-layer GCNConv + JK-cat + global_add_pool + MLP) on 8 Trainium2 cores.

Data-parallel over graphs: each core owns a contiguous range of graphs
(balanced by edge count).  Per layer:
  1. feat-major matmul  m'^T = W^T h^T  on PE, columns scaled by dinv
  2. transpose to node-major, write the core's slice to HBM, AllGather the
     full 51200-row message table (row-padded to 128 fp16 elems = 256B)
  3. dma_gather (gpsimd CounterMachine op, int16 idx, two half-tables)
     pulls each in-edge's source row into an SBUF edge buffer
  4. PE selection matmuls (lhsT = edge chunk [128,96], rhs = one-hot S
     [128,128] built by DVE is_equal) accumulate per-dst sums in PSUM,
     feat-major
  5. finalize on DVE: + self-loop column, x dinv, + bias, relu -> h^T fp16
Pooling = per-column-chunk selection matmuls against graph-id one-hots;
JK 'cat' = 3 accumulated [96,96] matmuls; MLP feat-major; host transposes.

Node rows are permuted per-core so every (tile, table-half) has a uniform
number of 128-edge chunks across all 50 tiles and 8 cores (single SPMD
program), with ~3% padding.
"""

import numpy as np
import ml_dtypes

import concourse.bacc as bacc
import concourse.tile as tile
import concourse.mybir as mybir
from concourse import bass_utils
from concourse.masks import make_identity

# problem constants
N_NODES = 50000
N_EDGES = 800000
N_GRAPHS = 500
IN_DIM = 128
HID = 96
OUT_DIM = 64

# sharding constants
CORES = 8
S = 6400                   # padded rows per core
NT = 50                    # dst tiles of 128 rows per core
HALF_ROWS = CORES // 2 * S  # 25600, rows per int16-indexable half-table
EP = 128                   # padded table row elems (fp16 -> 256B)
GMAX = 80                  # max graphs per core (padded)
TILE_GROUPS = [range(0, 7), range(7, 14), range(14, 21), range(21, 28),
               range(28, 34), range(34, 40), range(40, 45), range(45, 50)]

bf16 = mybir.dt.bfloat16
fp16 = mybir.dt.float16
f32 = mybir.dt.float32
i16 = mybir.dt.int16

_f16 = ml_dtypes.float16 if not hasattr(np, "float16") else np.float16


def _wrap16(idx_flat):
    """dma_gather idx layout: idx i -> partition i%16, col i//16, replicated
    across the 8 q7 groups. idx_flat length must be a multiple of 16."""
    t = np.asarray(idx_flat, np.int16).reshape(-1, 16).T  # [16, n//16]
    return np.tile(t, (8, 1))  # [128, n//16]


def _balance_tiles(degA, degB, n_tiles, cap):
    """Assign nodes to n_tiles bins (<=cap nodes each), minimizing the max of
    per-bin sums of degA and degB.  Greedy on sorted total degree."""
    n = len(degA)
    order = np.argsort(-(degA + degB), kind="stable")
    sums = np.zeros((n_tiles, 2), np.int64)
    counts = np.zeros(n_tiles, np.int64)
    assign = np.empty(n, np.int32)
    for v in order:
        da, db = degA[v], degB[v]
        best, bestkey = -1, None
        for t in range(n_tiles):
            if counts[t] >= cap:
                continue
            key = (max(sums[t, 0] + da, sums[t, 1] + db),
                   sums[t, 0] + sums[t, 1])
            if bestkey is None or key < bestkey:
                best, bestkey = t, key
        assign[v] = best
        sums[best, 0] += da
        sums[best, 1] += db
        counts[best] += 1
    return assign


def _prepare(x, edge_index, batch):
    """Host-side graph preprocessing. Returns per-core input maps (minus
    weights) plus the layout metadata needed to build the program."""
    src = np.asarray(edge_index[0], np.int64)
    dst = np.asarray(edge_index[1], np.int64)
    batch = np.asarray(batch, np.int64)
    x = np.asarray(x, np.float32)

    deg = np.bincount(dst, minlength=N_NODES) + 1  # + self loop
    dinv = (1.0 / np.sqrt(deg)).astype(np.float32)

    gcounts = np.bincount(batch, minlength=N_GRAPHS)
    gnode_start = np.concatenate([[0], np.cumsum(gcounts)])
    # edges per graph (by dst's graph)
    epg = np.bincount(batch[dst], minlength=N_GRAPHS)
    cum = np.cumsum(epg)
    # contiguous graph split balancing edges
    bounds = [0]
    for k in range(1, CORES):
        bounds.append(int(np.searchsorted(cum, k * cum[-1] / CORES)))
    bounds.append(N_GRAPHS)
    core_graphs = [(bounds[c], bounds[c + 1]) for c in range(CORES)]
    core_nodes = [(int(gnode_start[g0]), int(gnode_start[g1]))
                  for g0, g1 in core_graphs]
    for n0, n1 in core_nodes:
        assert n1 - n0 <= S - 1, (n0, n1)

    owner = np.empty(N_NODES, np.int8)
    for c, (n0, n1) in enumerate(core_nodes):
        owner[n0:n1] = c
    src_half = (owner[src] >= CORES // 2)  # False -> half A

    # in-edge degree split per node
    degA = np.bincount(dst[~src_half], minlength=N_NODES)
    degB = np.bincount(dst[src_half], minlength=N_NODES)

    # per-core node permutation: tile-balanced assignment
    rowof = np.full(N_NODES, -1, np.int64)   # node -> global table row
    q_of = np.full(N_NODES, -1, np.int64)    # node -> local column q
    tiles_meta = []
    for c, (n0, n1) in enumerate(core_nodes):
        nodes = np.arange(n0, n1)
        assign = _balance_tiles(degA[n0:n1], degB[n0:n1], NT, 128)
        q = np.full(n1 - n0, -1, np.int64)
        for t in range(NT):
            members = np.where(assign == t)[0]
            q[members] = t * 128 + np.arange(len(members))
        q_of[nodes] = q
        rowof[nodes] = c * S + q
        tiles_meta.append(assign)

    # per (core, tile, half) edge lists
    e_core = owner[dst]
    e_q = q_of[dst]
    e_tile = e_q // 128
    e_p = (e_q % 128).astype(np.int16)
    e_row16 = (rowof[src] - src_half * HALF_ROWS).astype(np.int16)
    assert e_row16.min() >= 0

    lists = [[[None, None] for _ in range(NT)] for _ in range(CORES)]
    order = np.lexsort((e_q, src_half, e_tile, e_core))
    sc, st, sh = e_core[order], e_tile[order], src_half[order]
    srow, sp = e_row16[order], e_p[order]
    # boundaries
    key = ((sc.astype(np.int64) * NT + st) * 2 + sh)
    cuts = np.concatenate([[0], np.where(np.diff(key))[0] + 1, [len(key)]])
    for a, b in zip(cuts[:-1], cuts[1:]):
        c, t, h = int(sc[a]), int(st[a]), int(sh[a])
        lists[c][t][h] = (srow[a:b], sp[a:b])

    # uniform chunk counts
    def nchunks(lst):
        return 0 if lst is None else (len(lst[0]) + 127) // 128
    TCA = max(1, max(nchunks(lists[c][t][0])
                     for c in range(CORES) for t in range(NT)))
    TCB = max(1, max(nchunks(lists[c][t][1])
                     for c in range(CORES) for t in range(NT)))

    NCH = NT * (TCA + TCB)
    in_maps = []
    for c in range(CORES):
        n0, n1 = core_nodes[c]
        g0, g1 = core_graphs[c]
        nodes = np.arange(n0, n1)
        q = q_of[nodes]

        # hxT [IN_DIM, S] fp16 (columns q)
        hxT = np.zeros((IN_DIM, S), _f16)
        hxT[:, q] = x[nodes].T.astype(_f16)

        dinv_q = np.zeros((1, S), _f16)
        dinv_q[0, q] = dinv[nodes].astype(_f16)
        dinv_q = np.broadcast_to(dinv_q, (128, S)).copy()

        batchf = np.full((128, NT), -1.0, _f16)
        batchf[q % 128, q // 128] = (batch[nodes] - g0).astype(_f16)

        cnt = np.zeros((1, GMAX), np.float32)
        cnt[0, :g1 - g0] = gcounts[g0:g1]
        cnt = np.broadcast_to(cnt, (128, GMAX)).copy()

        # gather index stream + dstloc, chunk order: per group: A-run, B-run
        dstloc = np.full((128, NCH), -1.0, _f16)
        gcolsA, gcolsB = [], []
        ch_seq = 0
        for grp in TILE_GROUPS:
            for h, tc_h, gcols in ((0, TCA, gcolsA), (1, TCB, gcolsB)):
                idx_run = np.zeros(len(grp) * tc_h * 128, np.int16)
                pos = 0
                for t in grp:
                    lst = lists[c][t][h]
                    rows = lst[0] if lst is not None else np.zeros(0, np.int16)
                    ps = lst[1] if lst is not None else np.zeros(0, np.int16)
                    npad = tc_h * 128
                    assert len(rows) <= npad
                    idx_run[pos:pos + len(rows)] = rows
                    pos += npad
                    # dstloc for these chunks
                    base_ch = ch_seq + (t - grp[0]) * tc_h if False else None
                    del base_ch
                    for j in range(tc_h):
                        a, b = j * 128, min((j + 1) * 128, len(rows))
                        if a < len(rows):
                            dstloc[:b - a, ch_seq] = ps[a:b].astype(_f16)
                        ch_seq += 1
                gcols.append(_wrap16(idx_run))
        assert ch_seq == NCH
        gidxA = np.concatenate(gcolsA, axis=1)
        gidxB = np.concatenate(gcolsB, axis=1)

        in_maps.append({
            "hxT": hxT,
            "dinvq": dinv_q,
            "batchf": batchf,
            "cnt": cnt,
            "gidxA": np.ascontiguousarray(gidxA),
            "gidxB": np.ascontiguousarray(gidxB),
            "dstloc": dstloc,
        })

    meta = {
        "TCA": TCA, "TCB": TCB, "NCH": NCH,
        "core_graphs": core_graphs,
    }
    return in_maps, meta


def _weights_inputs(W0, b0, W1, b1, W2, b2, Wjk, bjk, Wm1, bm1, Wm2, bm2):
    def f16(a):
        return np.ascontiguousarray(np.asarray(a, np.float32).astype(_f16))

    def col(v):
        return np.ascontiguousarray(
            np.asarray(v, np.float32).reshape(-1, 1))

    Wjk = np.asarray(Wjk, np.float32)
    iota = np.broadcast_to(np.arange(128, dtype=_f16).reshape(1, 128),
                           (128, 128)).copy()
    return {
        "W0": f16(W0), "W1": f16(W1), "W2": f16(W2),
        "Wjk1": f16(Wjk[:HID]), "Wjk2": f16(Wjk[HID:2 * HID]),
        "Wjk3": f16(Wjk[2 * HID:]),
        "Wm1": f16(Wm1), "Wm2": f16(Wm2),
        "b0": col(b0), "b1": col(b1), "b2": col(b2),
        "bjk": col(bjk), "bm1": col(bm1), "bm2": col(bm2),
        "iota": np.ascontiguousarray(iota),
    }


def build_program(TCA, TCB, collective=True, phase=5):
    NCH = NT * (TCA + TCB)
    nc = bacc.Bacc("TRN2", target_bir_lowering=False, debug=False,
                   num_devices=CORES)

    # I/O
    d_hxT = nc.dram_tensor("hxT", [IN_DIM, S], fp16, kind="ExternalInput")
    d_dinvq = nc.dram_tensor("dinvq", [128, S], fp16, kind="ExternalInput")
    d_batchf = nc.dram_tensor("batchf", [128, NT], fp16, kind="ExternalInput")
    d_cnt = nc.dram_tensor("cnt", [128, GMAX], f32, kind="ExternalInput")
    ncolsA = NT * TCA * 128 // 16
    ncolsB = NT * TCB * 128 // 16
    d_gidxA = nc.dram_tensor("gidxA", [128, ncolsA], i16, kind="ExternalInput")
    d_gidxB = nc.dram_tensor("gidxB", [128, ncolsB], i16, kind="ExternalInput")
    d_dstloc = nc.dram_tensor("dstloc", [128, NCH], fp16, kind="ExternalInput")
    d_W = {}
    for nm, shape in [("W0", [IN_DIM, HID]), ("W1", [HID, HID]),
                      ("W2", [HID, HID]), ("Wjk1", [HID, HID]),
                      ("Wjk2", [HID, HID]), ("Wjk3", [HID, HID]),
                      ("Wm1", [HID, HID]), ("Wm2", [HID, OUT_DIM])]:
        d_W[nm] = nc.dram_tensor(nm, shape, fp16, kind="ExternalInput")
    for nm, rows in [("b0", HID), ("b1", HID), ("b2", HID), ("bjk", HID),
                     ("bm1", HID), ("bm2", OUT_DIM)]:
        d_W[nm] = nc.dram_tensor(nm, [rows, 1], f32, kind="ExternalInput")
    d_iota = nc.dram_tensor("iota", [128, 128], fp16, kind="ExternalInput")
    d_out = nc.dram_tensor("predT", [OUT_DIM, GMAX], f32,
                           kind="ExternalOutput")

    with tile.TileContext(nc) as tc:
        _build_body(nc, tc, locals(), TCA, TCB, collective, phase)
    nc.compile()
    return nc


def _build_body(nc, tc, d, TCA, TCB, collective, phase=5):
    from contextlib import ExitStack
    d_W = d["d_W"]
    NCH = NT * (TCA + TCB)

    with ExitStack() as ctx:
        dram = ctx.enter_context(tc.tile_pool(name="dram", bufs=1,
                                              space="DRAM"))
        cst = ctx.enter_context(tc.tile_pool(name="cst", bufs=1))
        hp = ctx.enter_context(tc.tile_pool(name="hp", bufs=1))
        ebA = ctx.enter_context(tc.tile_pool(name="ebA", bufs=2))
        ebB = ctx.enter_context(tc.tile_pool(name="ebB", bufs=2))
        sp = ctx.enter_context(tc.tile_pool(name="sp", bufs=4))
        fin = ctx.enter_context(tc.tile_pool(name="fin", bufs=3))

        agin = dram.tile([S, EP], fp16)
        tables = [dram.tile([CORES * S, EP], fp16,
                            name=f"table{i}") for i in range(3)]

        # ---- load constants
        def load(name, shape, dt):
            t = cst.tile(shape, dt, name=f"c_{name}")
            nc.sync.dma_start(out=t[:], in_=d[f"d_{name}"][:]
                              if f"d_{name}" in d else d_W[name][:])
            return t

        W = {nm: load(nm, [IN_DIM if nm == "W0" else HID,
                           OUT_DIM if nm == "Wm2" else HID], fp16)
             for nm in ["W0", "W1", "W2", "Wjk1", "Wjk2", "Wjk3", "Wm1",
                        "Wm2"]}
        B = {nm: load(nm, [OUT_DIM if nm == "bm2" else HID, 1], f32)
             for nm in ["b0", "b1", "b2", "bjk", "bm1", "bm2"]}
        hxT = load("hxT", [IN_DIM, S], fp16)
        dinvq = load("dinvq", [128, S], fp16)
        batchf = load("batchf", [128, NT], fp16)
        cnt = load("cnt", [128, GMAX], f32)
        # per-gather-instruction dense idx tiles (dma_gather ucode wants
        # offset-0 dense [128, n/16] blocks)
        gidx_tiles = {}
        colA = colB = 0
        for gi, grp in enumerate(TILE_GROUPS):
            ng = len(grp)
            for half, tc_h in ((0, TCA), (1, TCB)):
                ncol = ng * tc_h * 8
                t = cst.tile([128, ncol], i16, name=f"gx{gi}_{half}")
                src = d["d_gidxA"] if half == 0 else d["d_gidxB"]
                c0 = colA if half == 0 else colB
                nc.sync.dma_start(out=t[:], in_=src[:, c0:c0 + ncol])
                gidx_tiles[(gi, half)] = t
                if half == 0:
                    colA += ncol
                else:
                    colB += ncol
        dstloc = load("dstloc", [128, NCH], fp16)
        iota = load("iota", [128, 128], fp16)

        ident = cst.tile([128, 128], fp16, name="ident")
        make_identity(nc, ident[:])

        stage = cst.tile([128, NT, EP], fp16, name="stage")
        nc.vector.memset(stage[:], 0)

        hTs = []
        for li in range(3):
            hTs.append(hp.tile([HID, S], fp16, name=f"h{li + 1}T"))
        mT = hp.tile([HID, S], fp16, name="mT")

        # ================= layers =================
        lay_ctx = ExitStack()
        pmm = lay_ctx.enter_context(tc.tile_pool(name="pmm", bufs=2,
                                                 space="PSUM"))
        ptr = lay_ctx.enter_context(tc.tile_pool(name="ptr", bufs=2,
                                                 space="PSUM"))
        pagg = lay_ctx.enter_context(tc.tile_pool(name="pagg", bufs=4,
                                                  space="PSUM"))
        for li, (Wl, bl) in enumerate([(W["W0"], B["b0"]),
                                       (W["W1"], B["b1"]),
                                       (W["W2"], B["b2"])]):
            hprev = hxT if li == 0 else hTs[li - 1]
            table = tables[li]

            # m'^T = W^T hprev ; scale columns by dinv
            for k0 in range(0, S, 512):
                k1 = min(k0 + 512, S)
                pm = pmm.tile([HID, 512], f32, name="pm")
                nc.tensor.matmul(out=pm[:, :k1 - k0], lhsT=Wl[:],
                                 rhs=hprev[:, k0:k1], start=True, stop=True)
                nc.vector.tensor_tensor(
                    out=mT[:, k0:k1], in0=pm[:, :k1 - k0],
                    in1=dinvq[:HID, k0:k1],
                    op=mybir.AluOpType.mult)

            # transpose to node-major staging and write AG input
            for t in range(NT):
                pt = ptr.tile([128, HID], fp16, tag="pt", name="pt")
                nc.tensor.transpose(out=pt[:], in_=mT[:, t * 128:(t + 1) * 128],
                                    identity=ident[:HID, :HID])
                nc.vector.tensor_copy(out=stage[:, t, :HID], in_=pt[:])
            # stage cell (p, t) = row t*128+p -> agin[(t*128+p), :]
            nc.sync.dma_start(
                out=agin[:].rearrange("(t p) e -> p t e", p=128),
                in_=stage[:])

            if collective:
                nc.gpsimd.collective_compute(
                    "AllGather", mybir.AluOpType.bypass,
                    replica_groups=[list(range(CORES))],
                    ins=[agin[:]], outs=[table[:]])
            else:
                for c in range(CORES):
                    nc.sync.dma_start(out=table[c * S:(c + 1) * S, :],
                                      in_=agin[:])

            # gather + aggregate per tile group
            if phase < 2:
                break
            ch_seq = 0
            for gi, grp in enumerate(TILE_GROUPS):
                ng = len(grp)
                bufA = ebA.tile([128, ng * TCA, EP], fp16, tag="ebA",
                                name=f"ebA{li}_{grp[0]}")
                bufB = ebB.tile([128, ng * TCB, EP], fp16, tag="ebB",
                                name=f"ebB{li}_{grp[0]}")
                nA = ng * TCA * 128
                nB = ng * TCB * 128
                nc.gpsimd.dma_gather(
                    out_ap=bufA[:], in_ap=table[0:HALF_ROWS, :],
                    idxs_ap=gidx_tiles[(gi, 0)][:],
                    num_idxs=nA, num_idxs_reg=nA, elem_size=EP,
                    single_packet=False)
                nc.gpsimd.dma_gather(
                    out_ap=bufB[:], in_ap=table[HALF_ROWS:, :],
                    idxs_ap=gidx_tiles[(gi, 1)][:],
                    num_idxs=nB, num_idxs_reg=nB, elem_size=EP,
                    single_packet=False)

                for ti, t in enumerate(grp):
                    if phase < 3:
                        continue
                    pa = pagg.tile([HID, 128], f32, name="pa")
                    nmm = TCA + TCB
                    mi = 0
                    for half, tc_h, buf in ((0, TCA, bufA), (1, TCB, bufB)):
                        ch0 = ch_seq + ti * tc_h if half == 0 else \
                            ch_seq + ng * TCA + ti * tc_h
                        for j4 in range(0, tc_h, 4):
                            jn = min(4, tc_h - j4)
                            s4 = sp.tile([128, 4, 128], fp16, tag="s4",
                                         name=f"s4_{li}_{t}_{half}_{j4}")
                            nc.vector.tensor_tensor(
                                out=s4[:, :jn, :],
                                in0=dstloc[:, ch0 + j4:ch0 + j4 + jn]
                                .unsqueeze(2).to_broadcast([128, jn, 128]),
                                in1=iota[:].unsqueeze(1)
                                .to_broadcast([128, jn, 128]),
                                op=mybir.AluOpType.is_equal)
                            for j in range(jn):
                                lidx = (ti * tc_h + j4 + j)
                                nc.tensor.matmul(
                                    out=pa[:],
                                    lhsT=buf[:, lidx, :HID],
                                    rhs=s4[:, j, :],
                                    start=(mi == 0), stop=(mi == nmm - 1))
                                mi += 1
                    # finalize tile t
                    cols = slice(t * 128, (t + 1) * 128)
                    f1 = fin.tile([HID, 128], f32, tag="f1", name=f"f1_{li}_{t}")
                    nc.vector.tensor_tensor(out=f1[:], in0=pa[:],
                                            in1=mT[:, cols],
                                            op=mybir.AluOpType.add)
                    nc.vector.tensor_tensor(
                        out=f1[:], in0=f1[:],
                        in1=dinvq[:HID, cols],
                        op=mybir.AluOpType.mult)
                    nc.vector.tensor_tensor(
                        out=f1[:], in0=f1[:],
                        in1=bl[:].to_broadcast([HID, 128]),
                        op=mybir.AluOpType.add)
                    nc.vector.tensor_scalar_max(hTs[li][:, cols], f1[:], 0.0)
                ch_seq += ng * (TCA + TCB)
            if phase >= 3:
                assert ch_seq == NCH
            if phase < 4:
                break

        # ================= pooling =================
        lay_ctx.close()
        if phase < 5:
            stub = fin.tile([OUT_DIM, GMAX], f32, tag="ot", name="otstub")
            nc.vector.memset(stub[:], 0)
            nc.sync.dma_start(out=d["d_out"][:], in_=stub[:])
            return
        pool_ctx = ExitStack()
        ptr = pool_ctx.enter_context(tc.tile_pool(name="ptr2", bufs=2,
                                                  space="PSUM"))
        ppool = pool_ctx.enter_context(tc.tile_pool(name="ppool", bufs=1,
                                                    space="PSUM"))
        pmisc = pool_ctx.enter_context(tc.tile_pool(name="pmisc", bufs=1,
                                                    space="PSUM"))
        pe = [ppool.tile([GMAX, HID], f32, name=f"pe{li}") for li in range(3)]
        for t in range(NT):
            pc = sp.tile([128, GMAX], fp16, tag="pc", name=f"pc{t}")
            nc.vector.tensor_tensor(
                out=pc[:],
                in0=batchf[:, t:t + 1].to_broadcast([128, GMAX]),
                in1=iota[:, :GMAX],
                op=mybir.AluOpType.is_equal)
            for li in range(3):
                pt = ptr.tile([128, HID], fp16, tag="pt", name="pt")
                nc.tensor.transpose(out=pt[:],
                                    in_=hTs[li][:, t * 128:(t + 1) * 128],
                                    identity=ident[:HID, :HID])
                ntile = fin.tile([128, HID], fp16, tag="nt",
                                 name=f"nt{t}_{li}")
                nc.vector.tensor_copy(out=ntile[:], in_=pt[:])
                nc.tensor.matmul(out=pe[li][:], lhsT=pc[:], rhs=ntile[:],
                                 start=(t == 0), stop=(t == NT - 1))

        # JK: emb_o^T = sum_l Wjk_l^T emb_l^T + bjk (x) cnt
        pjk = pmisc.tile([HID, GMAX], f32, name="pjk")
        for li, wn in enumerate(["Wjk1", "Wjk2", "Wjk3"]):
            es = fin.tile([GMAX, HID], fp16, tag="es", name=f"es{li}")
            nc.vector.tensor_copy(out=es[:], in_=pe[li][:])
            petr = ptr.tile([HID, GMAX], fp16, tag="pt", name="petr")
            nc.tensor.transpose(out=petr[:], in_=es[:],
                                identity=ident[:GMAX, :GMAX])
            ebT = fin.tile([HID, GMAX], fp16, tag="ebT", name=f"ebT{li}")
            nc.vector.tensor_copy(out=ebT[:], in_=petr[:])
            nc.tensor.matmul(out=pjk[:], lhsT=W[wn][:], rhs=ebT[:],
                             start=(li == 0), stop=(li == 2))
        bterm = fin.tile([HID, GMAX], f32, tag="bterm", name="bterm")
        nc.vector.tensor_tensor(out=bterm[:],
                                in0=B["bjk"][:].to_broadcast([HID, GMAX]),
                                in1=cnt[:HID, :],
                                op=mybir.AluOpType.mult)
        m1 = fin.tile([HID, GMAX], fp16, tag="m1", name="m1")
        nc.vector.tensor_tensor(out=m1[:], in0=pjk[:], in1=bterm[:],
                                op=mybir.AluOpType.add)

        # MLP
        p1 = pmisc.tile([HID, GMAX], f32, name="p1")
        nc.tensor.matmul(out=p1[:], lhsT=W["Wm1"][:], rhs=m1[:],
                         start=True, stop=True)
        r1f = fin.tile([HID, GMAX], f32, tag="r1f", name="r1f")
        nc.vector.tensor_tensor(out=r1f[:], in0=p1[:],
                                in1=B["bm1"][:].to_broadcast([HID, GMAX]),
                                op=mybir.AluOpType.add)
        r1 = fin.tile([HID, GMAX], fp16, tag="r1", name="r1")
        nc.vector.tensor_scalar_max(r1[:], r1f[:], 0.0)
        p2 = pmisc.tile([OUT_DIM, GMAX], f32, name="p2")
        nc.tensor.matmul(out=p2[:], lhsT=W["Wm2"][:], rhs=r1[:],
                         start=True, stop=True)
        ot = fin.tile([OUT_DIM, GMAX], f32, tag="ot", name="ot")
        nc.vector.tensor_tensor(out=ot[:], in0=p2[:],
                                in1=B["bm2"][:].to_broadcast([OUT_DIM, GMAX]),
                                op=mybir.AluOpType.add)
        nc.sync.dma_start(out=d["d_out"][:], in_=ot[:])
        pool_ctx.close()


_CACHE = {}


def _get_program(TCA, TCB, collective=True):
    key = (TCA, TCB, collective)
    if key not in _CACHE:
        _CACHE[key] = build_program(TCA, TCB, collective)
    return _CACHE[key]


def kernel(x, edge_index, batch, W0, b0, W1, b1, W2, b2, Wjk, bjk,
           Wm1, bm1, Wm2, bm2):
    in_maps, meta = _prepare(x, edge_index, batch)
    wmap = _weights_inputs(W0, b0, W1, b1, W2, b2, Wjk, bjk, Wm1, bm1,
                           Wm2, bm2)
    for m in in_maps:
        m.update(wmap)
    nc = _get_program(meta["TCA"], meta["TCB"], collective=True)
    r = bass_utils.run_bass_kernel_spmd(
        nc, in_maps, core_ids=list(range(CORES)), trace=False)
    out = np.zeros((N_GRAPHS, OUT_DIM), np.float32)
    for c, (g0, g1) in enumerate(meta["core_graphs"]):
        out[g0:g1] = r.results[c]["predT"][:, :g1 - g0].T
    return out


def build_null(TCA, TCB):
    """Same I/O signature, no work: dispatch-overhead baseline."""
    NCH = NT * (TCA + TCB)
    nc = bacc.Bacc("TRN2", target_bir_lowering=False, debug=False,
                   num_devices=CORES)
    nc.dram_tensor("hxT", [IN_DIM, S], fp16, kind="ExternalInput")
    nc.dram_tensor("dinvq", [128, S], fp16, kind="ExternalInput")
    nc.dram_tensor("batchf", [128, NT], fp16, kind="ExternalInput")
    nc.dram_tensor("cnt", [128, GMAX], f32, kind="ExternalInput")
    nc.dram_tensor("gidxA", [128, NT * TCA * 8], i16, kind="ExternalInput")
    nc.dram_tensor("gidxB", [128, NT * TCB * 8], i16, kind="ExternalInput")
    nc.dram_tensor("dstloc", [128, NCH], fp16, kind="ExternalInput")
    for nm, shape in [("W0", [IN_DIM, HID]), ("W1", [HID, HID]),
                      ("W2", [HID, HID]), ("Wjk1", [HID, HID]),
                      ("Wjk2", [HID, HID]), ("Wjk3", [HID, HID]),
                      ("Wm1", [HID, HID]), ("Wm2", [HID, OUT_DIM])]:
        nc.dram_tensor(nm, shape, fp16, kind="ExternalInput")
    for nm, rows in [("b0", HID), ("b1", HID), ("b2", HID), ("bjk", HID),
                     ("bm1", HID), ("bm2", OUT_DIM)]:
        nc.dram_tensor(nm, [rows, 1], f32, kind="ExternalInput")
    nc.dram_tensor("iota", [128, 128], fp16, kind="ExternalInput")
    d_out = nc.dram_tensor("predT", [OUT_DIM, GMAX], f32,
                           kind="ExternalOutput")
    with tile.TileContext(nc) as tc:
        with tc.tile_pool(name="sb", bufs=1) as sb:
            z = sb.tile([OUT_DIM, GMAX], f32)
            nc.vector.memset(z[:], 0)
            nc.sync.dma_start(out=d_out[:], in_=z[:])
    nc.compile()
    return nc


# revision 17
# speedup vs baseline: 2957808.0000x; 2957808.0000x over previous
"""GCN (3-layer GCNConv + JK-cat + global_add_pool + MLP) on 8 Trainium2 cores.

Data-parallel over graphs: each core owns a contiguous range of graphs
(balanced by edge count).  Per layer:
  1. feat-major matmul  m'^T = W^T h^T  on PE, columns scaled by dinv
  2. transpose to node-major, write the core's slice to HBM, AllGather the
     full 51200-row message table (row-padded to 128 fp16 elems = 256B)
  3. dma_gather (gpsimd CounterMachine op, int16 idx, two half-tables)
     pulls each in-edge's source row into an SBUF edge buffer
  4. PE selection matmuls (lhsT = edge chunk [128,96], rhs = one-hot S
     [128,128] built by DVE is_equal) accumulate per-dst sums in PSUM,
     feat-major
  5. finalize on DVE: + self-loop column, x dinv, + bias, relu -> h^T fp16
Pooling = per-column-chunk selection matmuls against graph-id one-hots;
JK 'cat' = 3 accumulated [96,96] matmuls; MLP feat-major; host transposes.

Node rows are permuted per-core so every (tile, table-half) has a uniform
number of 128-edge chunks across all 50 tiles and 8 cores (single SPMD
program), with ~3% padding.
"""

import numpy as np
import ml_dtypes

import concourse.bacc as bacc
import concourse.tile as tile
import concourse.mybir as mybir
from concourse import bass_utils
from concourse.masks import make_identity

# problem constants
N_NODES = 50000
N_EDGES = 800000
N_GRAPHS = 500
IN_DIM = 128
HID = 96
OUT_DIM = 64

# sharding constants
CORES = 8
S = 6400                   # padded rows per core
NT = 50                    # dst tiles of 128 rows per core
HALF_ROWS = CORES // 2 * S  # 25600, rows per int16-indexable half-table
EP = 128                   # padded table row elems (fp16 -> 256B)
GMAX = 80                  # max graphs per core (padded)
TILE_GROUPS = [range(0, 7), range(7, 14), range(14, 21), range(21, 28),
               range(28, 34), range(34, 40), range(40, 45), range(45, 50)]

bf16 = mybir.dt.bfloat16
fp16 = mybir.dt.float16
f32 = mybir.dt.float32
i16 = mybir.dt.int16

_f16 = ml_dtypes.float16 if not hasattr(np, "float16") else np.float16


def _wrap16(idx_flat):
    """dma_gather idx layout: idx i -> partition i%16, col i//16, replicated
    across the 8 q7 groups. idx_flat length must be a multiple of 16."""
    t = np.asarray(idx_flat, np.int16).reshape(-1, 16).T  # [16, n//16]
    return np.tile(t, (8, 1))  # [128, n//16]


def _balance_tiles(degA, degB, n_tiles, cap):
    """Assign nodes to n_tiles bins (<=cap nodes each), minimizing the max of
    per-bin sums of degA and degB.  Greedy on sorted total degree."""
    n = len(degA)
    order = np.argsort(-(degA + degB), kind="stable")
    sums = np.zeros((n_tiles, 2), np.int64)
    counts = np.zeros(n_tiles, np.int64)
    assign = np.empty(n, np.int32)
    for v in order:
        da, db = degA[v], degB[v]
        best, bestkey = -1, None
        for t in range(n_tiles):
            if counts[t] >= cap:
                continue
            key = (max(sums[t, 0] + da, sums[t, 1] + db),
                   sums[t, 0] + sums[t, 1])
            if bestkey is None or key < bestkey:
                best, bestkey = t, key
        assign[v] = best
        sums[best, 0] += da
        sums[best, 1] += db
        counts[best] += 1
    return assign


def _prepare(x, edge_index, batch):
    """Host-side graph preprocessing. Returns per-core input maps (minus
    weights) plus the layout metadata needed to build the program."""
    src = np.asarray(edge_index[0], np.int64)
    dst = np.asarray(edge_index[1], np.int64)
    batch = np.asarray(batch, np.int64)
    x = np.asarray(x, np.float32)

    deg = np.bincount(dst, minlength=N_NODES) + 1  # + self loop
    dinv = (1.0 / np.sqrt(deg)).astype(np.float32)

    gcounts = np.bincount(batch, minlength=N_GRAPHS)
    gnode_start = np.concatenate([[0], np.cumsum(gcounts)])
    # edges per graph (by dst's graph)
    epg = np.bincount(batch[dst], minlength=N_GRAPHS)
    cum = np.cumsum(epg)
    # contiguous graph split balancing edges
    bounds = [0]
    for k in range(1, CORES):
        bounds.append(int(np.searchsorted(cum, k * cum[-1] / CORES)))
    bounds.append(N_GRAPHS)
    core_graphs = [(bounds[c], bounds[c + 1]) for c in range(CORES)]
    core_nodes = [(int(gnode_start[g0]), int(gnode_start[g1]))
                  for g0, g1 in core_graphs]
    for n0, n1 in core_nodes:
        assert n1 - n0 <= S - 1, (n0, n1)

    owner = np.empty(N_NODES, np.int8)
    for c, (n0, n1) in enumerate(core_nodes):
        owner[n0:n1] = c
    src_half = (owner[src] >= CORES // 2)  # False -> half A

    # in-edge degree split per node
    degA = np.bincount(dst[~src_half], minlength=N_NODES)
    degB = np.bincount(dst[src_half], minlength=N_NODES)

    # per-core node permutation: tile-balanced assignment
    rowof = np.full(N_NODES, -1, np.int64)   # node -> global table row
    q_of = np.full(N_NODES, -1, np.int64)    # node -> local column q
    tiles_meta = []
    for c, (n0, n1) in enumerate(core_nodes):
        nodes = np.arange(n0, n1)
        assign = _balance_tiles(degA[n0:n1], degB[n0:n1], NT, 128)
        q = np.full(n1 - n0, -1, np.int64)
        for t in range(NT):
            members = np.where(assign == t)[0]
            q[members] = t * 128 + np.arange(len(members))
        q_of[nodes] = q
        rowof[nodes] = c * S + q
        tiles_meta.append(assign)

    # per (core, tile, half) edge lists
    e_core = owner[dst]
    e_q = q_of[dst]
    e_tile = e_q // 128
    e_p = (e_q % 128).astype(np.int16)
    e_row16 = (rowof[src] - src_half * HALF_ROWS).astype(np.int16)
    assert e_row16.min() >= 0

    lists = [[[None, None] for _ in range(NT)] for _ in range(CORES)]
    order = np.lexsort((e_q, src_half, e_tile, e_core))
    sc, st, sh = e_core[order], e_tile[order], src_half[order]
    srow, sp = e_row16[order], e_p[order]
    # boundaries
    key = ((sc.astype(np.int64) * NT + st) * 2 + sh)
    cuts = np.concatenate([[0], np.where(np.diff(key))[0] + 1, [len(key)]])
    for a, b in zip(cuts[:-1], cuts[1:]):
        c, t, h = int(sc[a]), int(st[a]), int(sh[a])
        lists[c][t][h] = (srow[a:b], sp[a:b])

    # uniform chunk counts
    def nchunks(lst):
        return 0 if lst is None else (len(lst[0]) + 127) // 128
    TCA = max(1, max(nchunks(lists[c][t][0])
                     for c in range(CORES) for t in range(NT)))
    TCB = max(1, max(nchunks(lists[c][t][1])
                     for c in range(CORES) for t in range(NT)))

    NCH = NT * (TCA + TCB)
    in_maps = []
    for c in range(CORES):
        n0, n1 = core_nodes[c]
        g0, g1 = core_graphs[c]
        nodes = np.arange(n0, n1)
        q = q_of[nodes]

        # hxT [IN_DIM, S] fp16 (columns q)
        hxT = np.zeros((IN_DIM, S), _f16)
        hxT[:, q] = x[nodes].T.astype(_f16)

        dinv_q = np.zeros((1, S), _f16)
        dinv_q[0, q] = dinv[nodes].astype(_f16)
        dinv_q = np.broadcast_to(dinv_q, (128, S)).copy()

        batchf = np.full((128, NT), -1.0, _f16)
        batchf[q % 128, q // 128] = (batch[nodes] - g0).astype(_f16)

        cnt = np.zeros((1, GMAX), np.float32)
        cnt[0, :g1 - g0] = gcounts[g0:g1]
        cnt = np.broadcast_to(cnt, (128, GMAX)).copy()

        # gather index stream + dstloc, chunk order: per group: A-run, B-run
        dstloc = np.full((128, NCH), -1.0, _f16)
        gcolsA, gcolsB = [], []
        ch_seq = 0
        for grp in TILE_GROUPS:
            for h, tc_h, gcols in ((0, TCA, gcolsA), (1, TCB, gcolsB)):
                idx_run = np.zeros(len(grp) * tc_h * 128, np.int16)
                pos = 0
                for t in grp:
                    lst = lists[c][t][h]
                    rows = lst[0] if lst is not None else np.zeros(0, np.int16)
                    ps = lst[1] if lst is not None else np.zeros(0, np.int16)
                    npad = tc_h * 128
                    assert len(rows) <= npad
                    idx_run[pos:pos + len(rows)] = rows
                    pos += npad
                    # dstloc for these chunks
                    base_ch = ch_seq + (t - grp[0]) * tc_h if False else None
                    del base_ch
                    for j in range(tc_h):
                        a, b = j * 128, min((j + 1) * 128, len(rows))
                        if a < len(rows):
                            dstloc[:b - a, ch_seq] = ps[a:b].astype(_f16)
                        ch_seq += 1
                gcols.append(_wrap16(idx_run))
        assert ch_seq == NCH
        gidxA = np.concatenate(gcolsA, axis=1)
        gidxB = np.concatenate(gcolsB, axis=1)

        in_maps.append({
            "hxT": hxT,
            "dinvq": dinv_q,
            "batchf": batchf,
            "cnt": cnt,
            "gidxA": np.ascontiguousarray(gidxA),
            "gidxB": np.ascontiguousarray(gidxB),
            "dstloc": dstloc,
        })

    meta = {
        "TCA": TCA, "TCB": TCB, "NCH": NCH,
        "core_graphs": core_graphs,
    }
    return in_maps, meta


def _weights_inputs(W0, b0, W1, b1, W2, b2, Wjk, bjk, Wm1, bm1, Wm2, bm2):
    def f16(a):
        return np.ascontiguousarray(np.asarray(a, np.float32).astype(_f16))

    def col(v):
        return np.ascontiguousarray(
            np.asarray(v, np.float32).reshape(-1, 1))

    Wjk = np.asarray(Wjk, np.float32)
    iota = np.broadcast_to(np.arange(128, dtype=_f16).reshape(1, 128),
                           (128, 128)).copy()
    return {
        "W0": f16(W0), "W1": f16(W1), "W2": f16(W2),
        "Wjk1": f16(Wjk[:HID]), "Wjk2": f16(Wjk[HID:2 * HID]),
        "Wjk3": f16(Wjk[2 * HID:]),
        "Wm1": f16(Wm1), "Wm2": f16(Wm2),
        "b0": col(b0), "b1": col(b1), "b2": col(b2),
        "bjk": col(bjk), "bm1": col(bm1), "bm2": col(bm2),
        "iota": np.ascontiguousarray(iota),
    }


def build_program(TCA, TCB, collective=True, phase=5):
    NCH = NT * (TCA + TCB)
    nc = bacc.Bacc("TRN2", target_bir_lowering=False, debug=False,
                   num_devices=CORES)

    # I/O
    d_hxT = nc.dram_tensor("hxT", [IN_DIM, S], fp16, kind="ExternalInput")
    d_dinvq = nc.dram_tensor("dinvq", [128, S], fp16, kind="ExternalInput")
    d_batchf = nc.dram_tensor("batchf", [128, NT], fp16, kind="ExternalInput")
    d_cnt = nc.dram_tensor("cnt", [128, GMAX], f32, kind="ExternalInput")
    ncolsA = NT * TCA * 128 // 16
    ncolsB = NT * TCB * 128 // 16
    d_gidxA = nc.dram_tensor("gidxA", [128, ncolsA], i16, kind="ExternalInput")
    d_gidxB = nc.dram_tensor("gidxB", [128, ncolsB], i16, kind="ExternalInput")
    d_dstloc = nc.dram_tensor("dstloc", [128, NCH], fp16, kind="ExternalInput")
    d_W = {}
    for nm, shape in [("W0", [IN_DIM, HID]), ("W1", [HID, HID]),
                      ("W2", [HID, HID]), ("Wjk1", [HID, HID]),
                      ("Wjk2", [HID, HID]), ("Wjk3", [HID, HID]),
                      ("Wm1", [HID, HID]), ("Wm2", [HID, OUT_DIM])]:
        d_W[nm] = nc.dram_tensor(nm, shape, fp16, kind="ExternalInput")
    for nm, rows in [("b0", HID), ("b1", HID), ("b2", HID), ("bjk", HID),
                     ("bm1", HID), ("bm2", OUT_DIM)]:
        d_W[nm] = nc.dram_tensor(nm, [rows, 1], f32, kind="ExternalInput")
    d_iota = nc.dram_tensor("iota", [128, 128], fp16, kind="ExternalInput")
    d_out = nc.dram_tensor("predT", [OUT_DIM, GMAX], f32,
                           kind="ExternalOutput")

    with tile.TileContext(nc) as tc:
        _build_body(nc, tc, locals(), TCA, TCB, collective, phase)
    nc.compile()
    return nc


def _build_body(nc, tc, d, TCA, TCB, collective, phase=5):
    from contextlib import ExitStack
    d_W = d["d_W"]
    NCH = NT * (TCA + TCB)

    with ExitStack() as ctx:
        dram = ctx.enter_context(tc.tile_pool(name="dram", bufs=1,
                                              space="DRAM"))
        cst = ctx.enter_context(tc.tile_pool(name="cst", bufs=1))
        hp = ctx.enter_context(tc.tile_pool(name="hp", bufs=1))
        ebA = ctx.enter_context(tc.tile_pool(name="ebA", bufs=2))
        ebB = ctx.enter_context(tc.tile_pool(name="ebB", bufs=2))
        sp = ctx.enter_context(tc.tile_pool(name="sp", bufs=4))
        fin = ctx.enter_context(tc.tile_pool(name="fin", bufs=3))

        agin = dram.tile([S, EP], fp16)
        tables = [dram.tile([CORES * S, EP], fp16,
                            name=f"table{i}") for i in range(3)]

        # ---- load constants
        def load(name, shape, dt):
            t = cst.tile(shape, dt, name=f"c_{name}")
            nc.sync.dma_start(out=t[:], in_=d[f"d_{name}"][:]
                              if f"d_{name}" in d else d_W[name][:])
            return t

        W = {nm: load(nm, [IN_DIM if nm == "W0" else HID,
                           OUT_DIM if nm == "Wm2" else HID], fp16)
             for nm in ["W0", "W1", "W2", "Wjk1", "Wjk2", "Wjk3", "Wm1",
                        "Wm2"]}
        B = {nm: load(nm, [OUT_DIM if nm == "bm2" else HID, 1], f32)
             for nm in ["b0", "b1", "b2", "bjk", "bm1", "bm2"]}
        hxT = load("hxT", [IN_DIM, S], fp16)
        dinvq = load("dinvq", [128, S], fp16)
        batchf = load("batchf", [128, NT], fp16)
        cnt = load("cnt", [128, GMAX], f32)
        # per-gather-instruction dense idx tiles (dma_gather ucode wants
        # offset-0 dense [128, n/16] blocks)
        gidx_tiles = {}
        colA = colB = 0
        for gi, grp in enumerate(TILE_GROUPS):
            ng = len(grp)
            for half, tc_h in ((0, TCA), (1, TCB)):
                ncol = ng * tc_h * 8
                t = cst.tile([128, ncol], i16, name=f"gx{gi}_{half}")
                src = d["d_gidxA"] if half == 0 else d["d_gidxB"]
                c0 = colA if half == 0 else colB
                nc.sync.dma_start(out=t[:], in_=src[:, c0:c0 + ncol])
                gidx_tiles[(gi, half)] = t
                if half == 0:
                    colA += ncol
                else:
                    colB += ncol
        dstloc = load("dstloc", [128, NCH], fp16)
        iota = load("iota", [128, 128], fp16)

        ident = cst.tile([128, 128], fp16, name="ident")
        make_identity(nc, ident[:])

        stage = cst.tile([128, NT, EP], fp16, name="stage")
        nc.vector.memset(stage[:], 0)

        hTs = []
        for li in range(3):
            hTs.append(hp.tile([HID, S], fp16, name=f"h{li + 1}T"))
        mT = hp.tile([HID, S], fp16, name="mT")

        # ================= layers =================
        lay_ctx = ExitStack()
        pmm = lay_ctx.enter_context(tc.tile_pool(name="pmm", bufs=2,
                                                 space="PSUM"))
        ptr = lay_ctx.enter_context(tc.tile_pool(name="ptr", bufs=2,
                                                 space="PSUM"))
        pagg = lay_ctx.enter_context(tc.tile_pool(name="pagg", bufs=4,
                                                  space="PSUM"))
        for li, (Wl, bl) in enumerate([(W["W0"], B["b0"]),
                                       (W["W1"], B["b1"]),
                                       (W["W2"], B["b2"])]):
            hprev = hxT if li == 0 else hTs[li - 1]
            table = tables[li]

            # m'^T = W^T hprev ; scale columns by dinv
            for k0 in range(0, S, 512):
                k1 = min(k0 + 512, S)
                pm = pmm.tile([HID, 512], f32, name="pm")
                nc.tensor.matmul(out=pm[:, :k1 - k0], lhsT=Wl[:],
                                 rhs=hprev[:, k0:k1], start=True, stop=True)
                nc.vector.tensor_tensor(
                    out=mT[:, k0:k1], in0=pm[:, :k1 - k0],
                    in1=dinvq[:HID, k0:k1],
                    op=mybir.AluOpType.mult)

            # transpose to node-major staging and write AG input
            for t in range(NT):
                pt = ptr.tile([128, HID], fp16, tag="pt", name="pt")
                nc.tensor.transpose(out=pt[:], in_=mT[:, t * 128:(t + 1) * 128],
                                    identity=ident[:HID, :HID])
                nc.vector.tensor_copy(out=stage[:, t, :HID], in_=pt[:])
            # stage cell (p, t) = row t*128+p -> agin[(t*128+p), :]
            nc.sync.dma_start(
                out=agin[:].rearrange("(t p) e -> p t e", p=128),
                in_=stage[:])

            if collective:
                nc.gpsimd.collective_compute(
                    "AllGather", mybir.AluOpType.bypass,
                    replica_groups=[list(range(CORES))],
                    ins=[agin[:]], outs=[table[:]])
            else:
                for c in range(CORES):
                    nc.sync.dma_start(out=table[c * S:(c + 1) * S, :],
                                      in_=agin[:])

            # gather + aggregate per tile group
            if phase < 2:
                break
            ch_seq = 0
            for gi, grp in enumerate(TILE_GROUPS):
                ng = len(grp)
                bufA = ebA.tile([128, ng * TCA, EP], fp16, tag="ebA",
                                name=f"ebA{li}_{grp[0]}")
                bufB = ebB.tile([128, ng * TCB, EP], fp16, tag="ebB",
                                name=f"ebB{li}_{grp[0]}")
                nA = ng * TCA * 128
                nB = ng * TCB * 128
                nc.gpsimd.dma_gather(
                    out_ap=bufA[:], in_ap=table[0:HALF_ROWS, :],
                    idxs_ap=gidx_tiles[(gi, 0)][:],
                    num_idxs=nA, num_idxs_reg=nA, elem_size=EP,
                    single_packet=False)
                nc.gpsimd.dma_gather(
                    out_ap=bufB[:], in_ap=table[HALF_ROWS:, :],
                    idxs_ap=gidx_tiles[(gi, 1)][:],
                    num_idxs=nB, num_idxs_reg=nB, elem_size=EP,
                    single_packet=False)

                for ti, t in enumerate(grp):
                    if phase < 3:
                        continue
                    pa = pagg.tile([HID, 128], f32, name="pa")
                    nmm = TCA + TCB
                    mi = 0
                    for half, tc_h, buf in ((0, TCA, bufA), (1, TCB, bufB)):
                        ch0 = ch_seq + ti * tc_h if half == 0 else \
                            ch_seq + ng * TCA + ti * tc_h
                        for j4 in range(0, tc_h, 8):
                            jn = min(8, tc_h - j4)
                            s4 = sp.tile([128, 8, 128], fp16, tag="s4",
                                         name=f"s4_{li}_{t}_{half}_{j4}")
                            nc.vector.tensor_tensor(
                                out=s4[:, :jn, :],
                                in0=dstloc[:, ch0 + j4:ch0 + j4 + jn]
                                .unsqueeze(2).to_broadcast([128, jn, 128]),
                                in1=iota[:].unsqueeze(1)
                                .to_broadcast([128, jn, 128]),
                                op=mybir.AluOpType.is_equal)
                            for j in range(jn):
                                lidx = (ti * tc_h + j4 + j)
                                nc.tensor.matmul(
                                    out=pa[:],
                                    lhsT=buf[:, lidx, :HID],
                                    rhs=s4[:, j, :],
                                    start=(mi == 0), stop=(mi == nmm - 1))
                                mi += 1
                    # finalize tile t
                    cols = slice(t * 128, (t + 1) * 128)
                    f1 = fin.tile([HID, 128], f32, tag="f1", name=f"f1_{li}_{t}")
                    nc.vector.tensor_tensor(out=f1[:], in0=pa[:],
                                            in1=mT[:, cols],
                                            op=mybir.AluOpType.add)
                    nc.vector.tensor_tensor(
                        out=f1[:], in0=f1[:],
                        in1=dinvq[:HID, cols],
                        op=mybir.AluOpType.mult)
                    nc.vector.tensor_tensor(
                        out=f1[:], in0=f1[:],
                        in1=bl[:].to_broadcast([HID, 128]),
                        op=mybir.AluOpType.add)
                    nc.vector.tensor_scalar_max(hTs[li][:, cols], f1[:], 0.0)
                ch_seq += ng * (TCA + TCB)
            if phase >= 3:
                assert ch_seq == NCH
            if phase < 4:
                break

        # ================= pooling =================
        lay_ctx.close()
        if phase < 5:
            stub = fin.tile([OUT_DIM, GMAX], f32, tag="ot", name="otstub")
            nc.vector.memset(stub[:], 0)
            nc.sync.dma_start(out=d["d_out"][:], in_=stub[:])
            return
        pool_ctx = ExitStack()
        ptr = pool_ctx.enter_context(tc.tile_pool(name="ptr2", bufs=2,
                                                  space="PSUM"))
        ppool = pool_ctx.enter_context(tc.tile_pool(name="ppool", bufs=1,
                                                    space="PSUM"))
        pmisc = pool_ctx.enter_context(tc.tile_pool(name="pmisc", bufs=1,
                                                    space="PSUM"))
        pe = [ppool.tile([GMAX, HID], f32, name=f"pe{li}") for li in range(3)]
        for t in range(NT):
            pc = sp.tile([128, GMAX], fp16, tag="pc", name=f"pc{t}")
            nc.vector.tensor_tensor(
                out=pc[:],
                in0=batchf[:, t:t + 1].to_broadcast([128, GMAX]),
                in1=iota[:, :GMAX],
                op=mybir.AluOpType.is_equal)
            for li in range(3):
                pt = ptr.tile([128, HID], fp16, tag="pt", name="pt")
                nc.tensor.transpose(out=pt[:],
                                    in_=hTs[li][:, t * 128:(t + 1) * 128],
                                    identity=ident[:HID, :HID])
                ntile = fin.tile([128, HID], fp16, tag="nt",
                                 name=f"nt{t}_{li}")
                nc.vector.tensor_copy(out=ntile[:], in_=pt[:])
                nc.tensor.matmul(out=pe[li][:], lhsT=pc[:], rhs=ntile[:],
                                 start=(t == 0), stop=(t == NT - 1))

        # JK: emb_o^T = sum_l Wjk_l^T emb_l^T + bjk (x) cnt
        pjk = pmisc.tile([HID, GMAX], f32, name="pjk")
        for li, wn in enumerate(["Wjk1", "Wjk2", "Wjk3"]):
            es = fin.tile([GMAX, HID], fp16, tag="es", name=f"es{li}")
            nc.vector.tensor_copy(out=es[:], in_=pe[li][:])
            petr = ptr.tile([HID, GMAX], fp16, tag="pt", name="petr")
            nc.tensor.transpose(out=petr[:], in_=es[:],
                                identity=ident[:GMAX, :GMAX])
            ebT = fin.tile([HID, GMAX], fp16, tag="ebT", name=f"ebT{li}")
            nc.vector.tensor_copy(out=ebT[:], in_=petr[:])
            nc.tensor.matmul(out=pjk[:], lhsT=W[wn][:], rhs=ebT[:],
                             start=(li == 0), stop=(li == 2))
        bterm = fin.tile([HID, GMAX], f32, tag="bterm", name="bterm")
        nc.vector.tensor_tensor(out=bterm[:],
                                in0=B["bjk"][:].to_broadcast([HID, GMAX]),
                                in1=cnt[:HID, :],
                                op=mybir.AluOpType.mult)
        m1 = fin.tile([HID, GMAX], fp16, tag="m1", name="m1")
        nc.vector.tensor_tensor(out=m1[:], in0=pjk[:], in1=bterm[:],
                                op=mybir.AluOpType.add)

        # MLP
        p1 = pmisc.tile([HID, GMAX], f32, name="p1")
        nc.tensor.matmul(out=p1[:], lhsT=W["Wm1"][:], rhs=m1[:],
                         start=True, stop=True)
        r1f = fin.tile([HID, GMAX], f32, tag="r1f", name="r1f")
        nc.vector.tensor_tensor(out=r1f[:], in0=p1[:],
                                in1=B["bm1"][:].to_broadcast([HID, GMAX]),
                                op=mybir.AluOpType.add)
        r1 = fin.tile([HID, GMAX], fp16, tag="r1", name="r1")
        nc.vector.tensor_scalar_max(r1[:], r1f[:], 0.0)
        p2 = pmisc.tile([OUT_DIM, GMAX], f32, name="p2")
        nc.tensor.matmul(out=p2[:], lhsT=W["Wm2"][:], rhs=r1[:],
                         start=True, stop=True)
        ot = fin.tile([OUT_DIM, GMAX], f32, tag="ot", name="ot")
        nc.vector.tensor_tensor(out=ot[:], in0=p2[:],
                                in1=B["bm2"][:].to_broadcast([OUT_DIM, GMAX]),
                                op=mybir.AluOpType.add)
        nc.sync.dma_start(out=d["d_out"][:], in_=ot[:])
        pool_ctx.close()


_CACHE = {}


def _get_program(TCA, TCB, collective=True):
    key = (TCA, TCB, collective)
    if key not in _CACHE:
        _CACHE[key] = build_program(TCA, TCB, collective)
    return _CACHE[key]


def kernel(x, edge_index, batch, W0, b0, W1, b1, W2, b2, Wjk, bjk,
           Wm1, bm1, Wm2, bm2):
    in_maps, meta = _prepare(x, edge_index, batch)
    wmap = _weights_inputs(W0, b0, W1, b1, W2, b2, Wjk, bjk, Wm1, bm1,
                           Wm2, bm2)
    for m in in_maps:
        m.update(wmap)
    nc = _get_program(meta["TCA"], meta["TCB"], collective=True)
    r = bass_utils.run_bass_kernel_spmd(
        nc, in_maps, core_ids=list(range(CORES)), trace=False)
    out = np.zeros((N_GRAPHS, OUT_DIM), np.float32)
    for c, (g0, g1) in enumerate(meta["core_graphs"]):
        out[g0:g1] = r.results[c]["predT"][:, :g1 - g0].T
    return out


def build_null(TCA, TCB):
    """Same I/O signature, no work: dispatch-overhead baseline."""
    NCH = NT * (TCA + TCB)
    nc = bacc.Bacc("TRN2", target_bir_lowering=False, debug=False,
                   num_devices=CORES)
    nc.dram_tensor("hxT", [IN_DIM, S], fp16, kind="ExternalInput")
    nc.dram_tensor("dinvq", [128, S], fp16, kind="ExternalInput")
    nc.dram_tensor("batchf", [128, NT], fp16, kind="ExternalInput")
    nc.dram_tensor("cnt", [128, GMAX], f32, kind="ExternalInput")
    nc.dram_tensor("gidxA", [128, NT * TCA * 8], i16, kind="ExternalInput")
    nc.dram_tensor("gidxB", [128, NT * TCB * 8], i16, kind="ExternalInput")
    nc.dram_tensor("dstloc", [128, NCH], fp16, kind="ExternalInput")
    for nm, shape in [("W0", [IN_DIM, HID]), ("W1", [HID, HID]),
                      ("W2", [HID, HID]), ("Wjk1", [HID, HID]),
                      ("Wjk2", [HID, HID]), ("Wjk3", [HID, HID]),
                      ("Wm1", [HID, HID]), ("Wm2", [HID, OUT_DIM])]:
        nc.dram_tensor(nm, shape, fp16, kind="ExternalInput")
    for nm, rows in [("b0", HID), ("b1", HID), ("b2", HID), ("bjk", HID),
                     ("bm1", HID), ("bm2", OUT_DIM)]:
        nc.dram_tensor(nm, [rows, 1], f32, kind="ExternalInput")
    nc.dram_tensor("iota", [128, 128], fp16, kind="ExternalInput")
    d_out = nc.dram_tensor("predT", [OUT_DIM, GMAX], f32,
                           kind="ExternalOutput")
    with tile.TileContext(nc) as tc:
        with tc.tile_pool(name="sb", bufs=1) as sb:
            z = sb.tile([OUT_DIM, GMAX], f32)
            nc.vector.memset(z[:], 0)
            nc.sync.dma_start(out=d_out[:], in_=z[:])
    nc.compile()
    return nc
